# revision 12
# baseline (speedup 1.0000x reference)
"""DeepSeekV2 MoE layer on 8 trn2 NeuronCores (expert-parallel).

Strategy (v5):
  - Host: gate softmax + group-limited top-k routing -> per-expert token index
    lists and combine weights (control data only; all heavy FLOPs on device).
    Experts are load-balanced across cores (serpentine over counts) and each
    core's 4 expert slots get per-slot capacities (max over cores, ceil 128).
  - Device (SPMD over 8 cores, 4 expert slots each):
      A: per slot: transposed fp16 dma_gather (double-buffered, spread over
         4 SWDGE queues) -> mm1/mm3 fp16 -> silu*mul -> g[slot] in SBUF;
         zero-fill of the y accumulators is interleaved here (sync/scalar);
      B: column-PAIR major down-proj: for each 1024-wide column pair: all 4
         slots' mm2 + gate-scale + one 2KB-elem dma_scatter_add per slot
         (negative-index padding skips pad tokens), then ReduceScatter(add)
         for the pair -> the 2 RS's overlap the shared-expert phase;
      S: shared-expert intermediate for own 512-token slice (after B so the
         RS chain hides under it; first S1A iters run up-front as warmup
         filler while the first gather lands);
      C: shared out matmuls + add RS result -> out.
  - Host: concatenate 512-row slices -> [B, S, H].
"""
import sys

import numpy as np

sys.path.insert(0, "/opt/trn_rl_repo")

import concourse.bass as bass
import concourse.mybir as mybir
import concourse.tile as tile
from concourse import bacc
from concourse.bass_utils import run_bass_kernel_spmd

F32 = mybir.dt.float32
FP16 = mybir.dt.float16
I16 = mybir.dt.int16
AF = mybir.ActivationFunctionType
OP = mybir.AluOpType

N_GROUP, TOPK_GROUP, TOP_K = 8, 3, 6
NCORES = 8
S1A = 5  # shared-intermediate iters run before phase A (warmup filler)


def _routing(x, gate_w):
    T, E = x.shape[0], gate_w.shape[0]
    logits = (x @ gate_w.T).astype(np.float64)
    e = np.exp(logits - logits.max(-1, keepdims=True))
    scores = e / e.sum(-1, keepdims=True)
    per_group = E // N_GROUP
    group_scores = scores.reshape(T, N_GROUP, per_group).max(-1)
    order = np.argsort(-group_scores, axis=-1, kind="stable")
    group_mask = np.zeros((T, N_GROUP), bool)
    np.put_along_axis(group_mask, order[:, :TOPK_GROUP], True, axis=1)
    tmp = np.where(np.repeat(group_mask, per_group, axis=1), scores, 0.0)
    order_e = np.argsort(-tmp, axis=-1, kind="stable")
    topk_idx = order_e[:, :TOP_K]
    topk_w = np.take_along_axis(tmp, topk_idx, axis=1)
    topk_w = topk_w / (topk_w.sum(-1, keepdims=True) + 1e-20)
    combine = np.zeros((T, E), np.float32)
    np.put_along_axis(combine, topk_idx, topk_w.astype(np.float32), axis=1)
    return combine


def _chunks(cap):
    out, rem = [], cap
    while rem:
        if rem <= 512:
            out.append(rem)
            rem = 0
        elif rem == 640:
            out.append(384)
            rem = 256
        else:
            out.append(512)
            rem -= 512
    return out


def build_kernel(T, H, I, CAPS, CAPS16, SI, act=AF.Silu, compile_=True):
    EPC = len(CAPS)
    KT = H // 128         # H contraction tiles
    MT = I // 128         # I tiles
    NP = max(H // 1024, 1)  # column pairs
    PW = min(H, 1024)       # pair width
    NW = 512
    SIT = SI // 128       # shared-intermediate tiles
    TOUT = T // NCORES    # own token slice
    TS = TOUT // 128
    CAP0 = max(CAPS)
    CT0 = CAP0 // 128
    CHUNKS = [_chunks(c) for c in CAPS]
    ZBLK = (T + 128) // 128  # zero blocks per pair tensor

    nc = bacc.Bacc("TRN2")
    x16 = nc.dram_tensor("x16", [T, H], FP16, kind="ExternalInput")
    xTc = nc.dram_tensor("xTc", [128, KT * TOUT], FP16, kind="ExternalInput")
    w13 = nc.dram_tensor("w13", [EPC, MT, 128, KT * 256], FP16, kind="ExternalInput")
    w2b = nc.dram_tensor("w2b", [EPC, H // NW, 128, MT * NW], FP16,
                         kind="ExternalInput")
    sw13 = nc.dram_tensor("sw13", [SIT, 128, KT * 256], FP16, kind="ExternalInput")
    sw2b = nc.dram_tensor("sw2b", [H // NW, 128, SIT * NW], FP16,
                          kind="ExternalInput")
    idx = nc.dram_tensor("idx", [EPC, 128, CAP0 // 16], I16, kind="ExternalInput")
    idxs = nc.dram_tensor("idxs", [EPC, 128, CAP0 // 16], I16, kind="ExternalInput")
    gat = nc.dram_tensor("gat", [EPC, 128, CT0], F32, kind="ExternalInput")
    out = nc.dram_tensor("out", [TOUT, H], FP16, kind="ExternalOutput")

    y_p = [nc.dram_tensor(f"y_pair{p}", [T + 128, PW], FP16) for p in range(NP)]
    rs_p = [nc.dram_tensor(f"rs_pair{p}", [TOUT, PW], FP16) for p in range(NP)]

    with tile.TileContext(nc) as tc:
        with (
            tc.tile_pool(name="const", bufs=1) as const,
            tc.tile_pool(name="persist", bufs=1) as persist,
            tc.tile_pool(name="xgtp", bufs=2) as xgtp,
            tc.tile_pool(name="wstream", bufs=4) as wstream,
            tc.tile_pool(name="ybp", bufs=2) as ybp,
            tc.tile_pool(name="small", bufs=2) as small,
            tc.tile_pool(name="psum", bufs=2, space="PSUM") as psum,
        ):
            idx_sb = const.tile([128, EPC, CAP0 // 16], I16)
            nc.scalar.dma_start(idx_sb[:], idx.rearrange("e p c -> p e c"))
            idxs_sb = const.tile([128, EPC, CAP0 // 16], I16)
            nc.scalar.dma_start(idxs_sb[:], idxs.rearrange("e p c -> p e c"))
            gat_sb = const.tile([128, EPC, CT0], F32)
            nc.scalar.dma_start(gat_sb[:], gat.rearrange("e p c -> p e c"))
            # shared-expert input (own tokens, H-tiled on partitions)
            xtc_sb = persist.tile([128, KT, TOUT], FP16)
            nc.scalar.dma_start(xtc_sb[:], xTc[:])
            gs = persist.tile([128, SIT, TOUT], FP16)
            g_sl = [persist.tile([128, MT, CAPS16[j]], FP16, tag=f"g{j}",
                                 name=f"g{j}")
                    for j in range(EPC)]
            ztile = const.tile([128, NW], FP16)
            nc.vector.memset(ztile[:], 0.0)

            def shared_int(sm):
                s13 = xgtp.tile([128, KT, 256], FP16, tag="xg1", name="s13")
                nc.sync.dma_start(
                    s13[:], sw13[sm].rearrange("p (k c) -> p k c", c=256))
                p1 = psum.tile([128, 512], F32, tag="p1")
                p3 = psum.tile([128, 512], F32, tag="p3")
                for k in range(KT):
                    nc.tensor.matmul(p1[:, :TOUT], s13[:, k, :128], xtc_sb[:, k, :],
                                     start=(k == 0), stop=(k == KT - 1))
                for k in range(KT):
                    nc.tensor.matmul(p3[:, :TOUT], s13[:, k, 128:], xtc_sb[:, k, :],
                                     start=(k == 0), stop=(k == KT - 1))
                nc.scalar.activation(gs[:, sm, :], p1[:, :TOUT], act)
                nc.vector.tensor_tensor(gs[:, sm, :], gs[:, sm, :], p3[:, :TOUT],
                                        OP.mult)

            # warmup filler while the first gathers land
            for sm in range(S1A):
                shared_int(sm)

            def bcol(p, j):
                capj = CAPS[j]
                ctj = capj // 128
                w2a = wstream.tile([128, MT, NW], FP16, tag="w", name="w2a")
                nc.scalar.dma_start(
                    w2a[:], w2b[j, 2 * p].rearrange("p (k c) -> p k c", c=NW))
                w2c = wstream.tile([128, MT, NW], FP16, tag="w", name="w2c")
                nc.scalar.dma_start(
                    w2c[:], w2b[j, 2 * p + 1].rearrange("p (k c) -> p k c",
                                                        c=NW))
                yb = ybp.tile([128, ctj, PW], FP16, tag="yb", name="yb")
                for ct in range(ctj):
                    ctw = min(128, CAPS16[j] - ct * 128)
                    p4a = psum.tile([128, NW], F32, tag="p4a")
                    p4b = psum.tile([128, NW], F32, tag="p4b")
                    for k2 in range(MT):
                        nc.tensor.matmul(p4a[:ctw],
                                         g_sl[j][:, k2,
                                                 ct * 128:ct * 128 + ctw],
                                         w2a[:, k2, :],
                                         start=(k2 == 0), stop=(k2 == MT - 1))
                    for k2 in range(MT):
                        nc.tensor.matmul(p4b[:ctw],
                                         g_sl[j][:, k2,
                                                 ct * 128:ct * 128 + ctw],
                                         w2c[:, k2, :],
                                         start=(k2 == 0), stop=(k2 == MT - 1))
                    gbc = gat_sb[:, j, ct:ct + 1].to_broadcast([128, NW])
                    nc.vector.tensor_tensor(yb[:, ct, :NW], p4a[:], gbc,
                                            OP.mult)
                    nc.vector.tensor_tensor(yb[:, ct, NW:], p4b[:], gbc,
                                            OP.mult)
                nc.gpsimd.dma_scatter_add(
                    y_p[p][:], yb[:], idxs_sb[:, j, :capj // 16],
                    capj, capj, PW)

            def zfill(p, half):
                blocks = range(half * ((ZBLK + 1) // 2),
                               min(ZBLK, (half + 1) * ((ZBLK + 1) // 2)))
                for bi, b in enumerate(blocks):
                    eng = nc.sync if bi % 2 == 0 else nc.scalar
                    eng.dma_start(y_p[p][b * 128:(b + 1) * 128, :NW], ztile[:])
                    eng2 = nc.scalar if bi % 2 == 0 else nc.sync
                    eng2.dma_start(y_p[p][b * 128:(b + 1) * 128, NW:], ztile[:])

            # pair-0 accumulator zeroed up-front (its scatters start mid-A)
            zfill(0, 0)
            zfill(0, 1)

            # ---------------- phase A: gathers + up-proj -> g; pair-0 -------
            # down-proj + scatter interleaved per expert so RS(0) can start
            # right at the end of A.
            for j in range(EPC):
                xgt_c = []
                c0 = 0
                for ci, cw in enumerate(CHUNKS[j]):
                    xgt = xgtp.tile([128, KT, cw], FP16, tag=f"xg{ci}",
                                    name=f"xg{ci}")
                    nc.gpsimd.dma_gather(
                        xgt[:], x16[:],
                        idx_sb[:, j, c0 // 16:(c0 + cw) // 16],
                        cw, cw, H, transpose=True)
                    xgt_c.append(xgt)
                    c0 += cw
                for m in range(MT):
                    w13t = wstream.tile([128, KT, 256], FP16, tag="w")
                    nc.sync.dma_start(
                        w13t[:], w13[j, m].rearrange("p (k c) -> p k c", c=256))
                    c0 = 0
                    for ci, cw in enumerate(CHUNKS[j]):
                        # compute width trimmed to the 16-granular capacity;
                        # the g tail [cwc, cw) stays garbage -> zero gate ->
                        # scatters to the dummy row.
                        cwc = min(cw, CAPS16[j] - c0)
                        p1 = psum.tile([128, 512], F32, tag="p1")
                        p3 = psum.tile([128, 512], F32, tag="p3")
                        for k in range(KT):
                            nc.tensor.matmul(p1[:, :cwc], w13t[:, k, :128],
                                             xgt_c[ci][:, k, :cwc],
                                             start=(k == 0), stop=(k == KT - 1))
                        for k in range(KT):
                            nc.tensor.matmul(p3[:, :cwc], w13t[:, k, 128:],
                                             xgt_c[ci][:, k, :cwc],
                                             start=(k == 0), stop=(k == KT - 1))
                        nc.scalar.activation(g_sl[j][:, m, c0:c0 + cwc],
                                             p1[:, :cwc], act)
                        nc.vector.tensor_tensor(g_sl[j][:, m, c0:c0 + cwc],
                                                g_sl[j][:, m, c0:c0 + cwc],
                                                p3[:, :cwc], OP.mult)
                        c0 += cw
                # pair-0 down-proj for this expert, then pair-1 zero-fill
                bcol(0, j)
                if j < NP:
                    zfill(1, j)

            nc.gpsimd.collective_compute(
                "ReduceScatter", OP.add,
                replica_groups=[list(range(NCORES))],
                ins=[y_p[0][0:T, :]],
                outs=[rs_p[0][:]],
            )

            # ---------------- phase B: pair-1 down-proj + RS ----------------
            for j in range(EPC):
                bcol(1, j)
            nc.gpsimd.collective_compute(
                "ReduceScatter", OP.add,
                replica_groups=[list(range(NCORES))],
                ins=[y_p[1][0:T, :]],
                outs=[rs_p[1][:]],
            )

            # ---------------- phase S: rest of shared intermediate ----------
            for sm in range(S1A, SIT):
                shared_int(sm)

            # ---------------- phase C: shared out + combine with rs ---------
            for n in range(H // NW):
                sh = []
                for hh in range(2):
                    s2 = wstream.tile([128, SIT // 2, NW], FP16, tag="w")
                    nc.scalar.dma_start(
                        s2[:], sw2b[n].rearrange("p (k c) -> p k c", c=NW)
                        [:, hh * (SIT // 2):(hh + 1) * (SIT // 2), :])
                    sh.append(s2)
                for ts in range(TS):
                    po = psum.tile([128, NW], F32, tag="p4a")
                    for k2 in range(SIT):
                        nc.tensor.matmul(po[:], gs[:, k2, ts * 128:(ts + 1) * 128],
                                         sh[k2 // (SIT // 2)][:, k2 % (SIT // 2), :],
                                         start=(k2 == 0), stop=(k2 == SIT - 1))
                    rst = small.tile([128, NW], FP16, tag="rst")
                    nc.sync.dma_start(
                        rst[:],
                        rs_p[n // 2][ts * 128:(ts + 1) * 128,
                                     (n % 2) * NW:(n % 2 + 1) * NW])
                    ott = small.tile([128, NW], FP16, tag="ott")
                    nc.vector.tensor_tensor(ott[:], po[:], rst[:], OP.add)
                    nc.sync.dma_start(
                        out[ts * 128:(ts + 1) * 128, n * NW:(n + 1) * NW], ott[:])

    if compile_:
        nc.compile()
    else:
        nc.insert_library_loads()
    return nc


def host_prep(hidden_states, gate_weight, w1, w2, w3, sw1, sw2, sw3):
    B, S, H = hidden_states.shape
    T = B * S
    E, I = w1.shape[0], w1.shape[1]
    SI = sw1.shape[0]
    EPC = E // NCORES
    KT, MT, SIT = H // 128, I // 128, SI // 128
    N4 = max(H // 512, 1)
    NW = min(H, 512)
    TOUT = T // NCORES

    x = np.ascontiguousarray(hidden_states.reshape(T, H), dtype=np.float32)
    combine = _routing(x, gate_weight.astype(np.float32))
    tok_lists = [np.nonzero(combine[:, e])[0] for e in range(E)]
    counts = np.array([len(t) for t in tok_lists])

    # balance experts over cores: serpentine over count-sorted experts; slot s
    # on every core holds its s-th largest expert, so per-slot capacity is the
    # max over cores of that rank band.
    order = np.argsort(-counts, kind="stable")
    assign = np.zeros((NCORES, EPC), np.int64)
    for s in range(EPC):
        band = order[s * NCORES:(s + 1) * NCORES]
        if s % 2 == 1:
            band = band[::-1]
        for c in range(NCORES):
            assign[c, s] = band[c]
    CAPS = tuple(
        int(max(128, ((counts[assign[:, s]].max() + 127) // 128) * 128))
        for s in range(EPC))
    CAPS16 = tuple(
        int(max(128, ((counts[assign[:, s]].max() + 15) // 16) * 16))
        for s in range(EPC))
    CAP0 = max(CAPS)
    CT0 = CAP0 // 128

    x16 = x.astype(np.float16)
    xT = x.T  # [H, T] view

    s1 = sw1.T.reshape(KT, 128, SIT, 128).transpose(2, 1, 0, 3)
    s3 = sw3.T.reshape(KT, 128, SIT, 128).transpose(2, 1, 0, 3)
    sw13 = np.ascontiguousarray(
        np.concatenate([s1, s3], axis=-1).reshape(SIT, 128, -1), dtype=np.float16)
    sw2b = np.ascontiguousarray(
        sw2.T.reshape(SIT, 128, N4, NW).transpose(2, 1, 0, 3).reshape(N4, 128, -1),
        dtype=np.float16)

    in_maps = []
    for c in range(NCORES):
        els = list(assign[c])
        idx_np = np.zeros((EPC, 128, CAP0 // 16), np.int16)
        idxs_np = np.zeros((EPC, 128, CAP0 // 16), np.int16)
        gat_np = np.zeros((EPC, 128, CT0), np.float32)
        for j, e in enumerate(els):
            cap = CAPS[j]
            toks = tok_lists[e]
            a = np.zeros(cap, np.int16)
            a[:len(toks)] = toks
            idx_np[j, :, :cap // 16] = np.tile(a.reshape(cap // 16, 16).T, (8, 1))
            b2 = np.full(cap, T, np.int16)  # pad rows land on dummy row T
            b2[:len(toks)] = toks
            idxs_np[j, :, :cap // 16] = np.tile(b2.reshape(cap // 16, 16).T, (8, 1))
            gv = np.zeros(cap, np.float32)
            gv[:len(toks)] = combine[toks, e]
            gat_np[j, :, :cap // 128] = gv.reshape(cap // 128, 128).T
        w13c = np.empty((EPC, MT, 128, KT * 256), np.float16)
        w2c = np.empty((EPC, N4, 128, MT * NW), np.float16)
        for j, e in enumerate(els):
            a1 = w1[e].T.reshape(KT, 128, MT, 128).transpose(2, 1, 0, 3)
            a3 = w3[e].T.reshape(KT, 128, MT, 128).transpose(2, 1, 0, 3)
            w13c[j] = np.concatenate([a1, a3], axis=-1).reshape(MT, 128, -1)
            w2c[j] = (w2[e].T.reshape(MT, 128, N4, NW)
                      .transpose(2, 1, 0, 3).reshape(N4, 128, -1))
        xTc = np.ascontiguousarray(
            xT[:, c * TOUT:(c + 1) * TOUT].reshape(KT, 128, TOUT)
            .transpose(1, 0, 2).reshape(128, -1), dtype=np.float16)
        in_maps.append({
            "x16": x16, "xTc": xTc,
            "w13": w13c, "w2b": w2c,
            "sw13": sw13, "sw2b": sw2b,
            "idx": idx_np, "idxs": idxs_np, "gat": gat_np,
        })
    cfg = dict(T=T, H=H, I=I, CAPS=CAPS, CAPS16=CAPS16, SI=SI)
    return in_maps, cfg


def kernel(**inputs):
    inputs = {k: np.asarray(v) for k, v in inputs.items()}
    hs = inputs["hidden_states"]
    B, S, H = hs.shape
    in_maps, cfg = host_prep(
        hs, inputs["gate_weight"], inputs["w1"], inputs["w2"], inputs["w3"],
        inputs["sw1"], inputs["sw2"], inputs["sw3"])
    nc = build_kernel(**cfg)
    res = run_bass_kernel_spmd(nc, in_maps, list(range(NCORES)))
    y = np.concatenate([res.results[c]["out"] for c in range(NCORES)], axis=0)
    return y.reshape(B, S, H).astype(np.float32)


if __name__ == "__main__":
    pass


# revision 13
# speedup vs baseline: 1.0269x; 1.0269x over previous
"""DeepSeekV2 MoE layer on 8 trn2 NeuronCores (expert-parallel).

Strategy (v5):
  - Host: gate softmax + group-limited top-k routing -> per-expert token index
    lists and combine weights (control data only; all heavy FLOPs on device).
    Experts are load-balanced across cores (serpentine over counts) and each
    core's 4 expert slots get per-slot capacities (max over cores, ceil 128).
  - Device (SPMD over 8 cores, 4 expert slots each):
      A: per slot: transposed fp16 dma_gather (double-buffered, spread over
         4 SWDGE queues) -> mm1/mm3 fp16 -> silu*mul -> g[slot] in SBUF;
         zero-fill of the y accumulators is interleaved here (sync/scalar);
      B: column-PAIR major down-proj: for each 1024-wide column pair: all 4
         slots' mm2 + gate-scale + one 2KB-elem dma_scatter_add per slot
         (negative-index padding skips pad tokens), then ReduceScatter(add)
         for the pair -> the 2 RS's overlap the shared-expert phase;
      S: shared-expert intermediate for own 512-token slice (after B so the
         RS chain hides under it; first S1A iters run up-front as warmup
         filler while the first gather lands);
      C: shared out matmuls + add RS result -> out.
  - Host: concatenate 512-row slices -> [B, S, H].
"""
import sys

import numpy as np

sys.path.insert(0, "/opt/trn_rl_repo")

import concourse.bass as bass
import concourse.mybir as mybir
import concourse.tile as tile
from concourse import bacc
from concourse.bass_utils import run_bass_kernel_spmd

F32 = mybir.dt.float32
FP16 = mybir.dt.float16
I16 = mybir.dt.int16
AF = mybir.ActivationFunctionType
OP = mybir.AluOpType

N_GROUP, TOPK_GROUP, TOP_K = 8, 3, 6
NCORES = 8
S1A = 7  # shared-intermediate iters run before phase A (warmup filler)


def _routing(x, gate_w):
    T, E = x.shape[0], gate_w.shape[0]
    logits = (x @ gate_w.T).astype(np.float64)
    e = np.exp(logits - logits.max(-1, keepdims=True))
    scores = e / e.sum(-1, keepdims=True)
    per_group = E // N_GROUP
    group_scores = scores.reshape(T, N_GROUP, per_group).max(-1)
    order = np.argsort(-group_scores, axis=-1, kind="stable")
    group_mask = np.zeros((T, N_GROUP), bool)
    np.put_along_axis(group_mask, order[:, :TOPK_GROUP], True, axis=1)
    tmp = np.where(np.repeat(group_mask, per_group, axis=1), scores, 0.0)
    order_e = np.argsort(-tmp, axis=-1, kind="stable")
    topk_idx = order_e[:, :TOP_K]
    topk_w = np.take_along_axis(tmp, topk_idx, axis=1)
    topk_w = topk_w / (topk_w.sum(-1, keepdims=True) + 1e-20)
    combine = np.zeros((T, E), np.float32)
    np.put_along_axis(combine, topk_idx, topk_w.astype(np.float32), axis=1)
    return combine


def _chunks(cap):
    out, rem = [], cap
    while rem:
        if rem <= 512:
            out.append(rem)
            rem = 0
        elif rem == 640:
            out.append(384)
            rem = 256
        else:
            out.append(512)
            rem -= 512
    return out


def build_kernel(T, H, I, CAPS, CAPS16, SI, act=AF.Silu, compile_=True):
    EPC = len(CAPS)
    KT = H // 128         # H contraction tiles
    MT = I // 128         # I tiles
    NP = max(H // 1024, 1)  # column pairs
    PW = min(H, 1024)       # pair width
    NW = 512
    SIT = SI // 128       # shared-intermediate tiles
    TOUT = T // NCORES    # own token slice
    TS = TOUT // 128
    CAP0 = max(CAPS)
    CT0 = CAP0 // 128
    CHUNKS = [_chunks(c) for c in CAPS]
    ZBLK = (T + 128) // 128  # zero blocks per pair tensor

    nc = bacc.Bacc("TRN2")
    x16 = nc.dram_tensor("x16", [T, H], FP16, kind="ExternalInput")
    xTc = nc.dram_tensor("xTc", [128, KT * TOUT], FP16, kind="ExternalInput")
    w13 = nc.dram_tensor("w13", [EPC, MT, 128, KT * 256], FP16, kind="ExternalInput")
    w2b = nc.dram_tensor("w2b", [EPC, H // NW, 128, MT * NW], FP16,
                         kind="ExternalInput")
    sw13 = nc.dram_tensor("sw13", [SIT, 128, KT * 256], FP16, kind="ExternalInput")
    sw2b = nc.dram_tensor("sw2b", [H // NW, 128, SIT * NW], FP16,
                          kind="ExternalInput")
    idx = nc.dram_tensor("idx", [EPC, 128, CAP0 // 16], I16, kind="ExternalInput")
    idxs = nc.dram_tensor("idxs", [EPC, 128, CAP0 // 16], I16, kind="ExternalInput")
    gat = nc.dram_tensor("gat", [EPC, 128, CT0], F32, kind="ExternalInput")
    out = nc.dram_tensor("out", [TOUT, H], FP16, kind="ExternalOutput")

    y_p = [nc.dram_tensor(f"y_pair{p}", [T + 128, PW], FP16) for p in range(NP)]
    rs_p = [nc.dram_tensor(f"rs_pair{p}", [TOUT, PW], FP16) for p in range(NP)]

    with tile.TileContext(nc) as tc:
        with (
            tc.tile_pool(name="const", bufs=1) as const,
            tc.tile_pool(name="persist", bufs=1) as persist,
            tc.tile_pool(name="xgtp", bufs=2) as xgtp,
            tc.tile_pool(name="wstream", bufs=4) as wstream,
            tc.tile_pool(name="ybp", bufs=2) as ybp,
            tc.tile_pool(name="small", bufs=2) as small,
            tc.tile_pool(name="psum", bufs=2, space="PSUM") as psum,
        ):
            idx_sb = const.tile([128, EPC, CAP0 // 16], I16)
            nc.scalar.dma_start(idx_sb[:], idx.rearrange("e p c -> p e c"))
            idxs_sb = const.tile([128, EPC, CAP0 // 16], I16)
            nc.scalar.dma_start(idxs_sb[:], idxs.rearrange("e p c -> p e c"))
            gat_sb = const.tile([128, EPC, CT0], F32)
            nc.scalar.dma_start(gat_sb[:], gat.rearrange("e p c -> p e c"))
            # shared-expert input (own tokens, H-tiled on partitions)
            xtc_sb = persist.tile([128, KT, TOUT], FP16)
            nc.scalar.dma_start(xtc_sb[:], xTc[:])
            gs = persist.tile([128, SIT, TOUT], FP16)
            g_sl = [persist.tile([128, MT, CAPS16[j]], FP16, tag=f"g{j}",
                                 name=f"g{j}")
                    for j in range(EPC)]
            ztile = const.tile([128, NW], FP16)
            nc.vector.memset(ztile[:], 0.0)

            def shared_int(sm):
                # warmup iters stay on xg1 (xg0 must stay free for the first
                # gather); post-B iters alternate both tags and both HWDGE
                # rings for 4-deep prefetch under RS-induced HBM contention.
                tag = "xg1" if sm < S1A else f"xg{sm % 2}"
                eng = nc.sync if (sm < S1A or sm % 2 == 0) else nc.scalar
                s13 = xgtp.tile([128, KT, 256], FP16, tag=tag, name="s13")
                eng.dma_start(
                    s13[:], sw13[sm].rearrange("p (k c) -> p k c", c=256))
                p1 = psum.tile([128, 512], F32, tag="p1")
                p3 = psum.tile([128, 512], F32, tag="p3")
                for k in range(KT):
                    nc.tensor.matmul(p1[:, :TOUT], s13[:, k, :128], xtc_sb[:, k, :],
                                     start=(k == 0), stop=(k == KT - 1))
                for k in range(KT):
                    nc.tensor.matmul(p3[:, :TOUT], s13[:, k, 128:], xtc_sb[:, k, :],
                                     start=(k == 0), stop=(k == KT - 1))
                nc.scalar.activation(gs[:, sm, :], p1[:, :TOUT], act)
                nc.vector.tensor_tensor(gs[:, sm, :], gs[:, sm, :], p3[:, :TOUT],
                                        OP.mult)

            # warmup filler while the first gathers land
            for sm in range(S1A):
                shared_int(sm)

            def bcol(p, j):
                capj = CAPS[j]
                ctj = capj // 128
                w2a = wstream.tile([128, MT, NW], FP16, tag="w", name="w2a")
                nc.scalar.dma_start(
                    w2a[:], w2b[j, 2 * p].rearrange("p (k c) -> p k c", c=NW))
                w2c = wstream.tile([128, MT, NW], FP16, tag="w", name="w2c")
                nc.scalar.dma_start(
                    w2c[:], w2b[j, 2 * p + 1].rearrange("p (k c) -> p k c",
                                                        c=NW))
                yb = ybp.tile([128, ctj, PW], FP16, tag="yb", name="yb")
                for ct in range(ctj):
                    ctw = min(128, CAPS16[j] - ct * 128)
                    p4a = psum.tile([128, NW], F32, tag="p4a")
                    p4b = psum.tile([128, NW], F32, tag="p4b")
                    for k2 in range(MT):
                        nc.tensor.matmul(p4a[:ctw],
                                         g_sl[j][:, k2,
                                                 ct * 128:ct * 128 + ctw],
                                         w2a[:, k2, :],
                                         start=(k2 == 0), stop=(k2 == MT - 1))
                    for k2 in range(MT):
                        nc.tensor.matmul(p4b[:ctw],
                                         g_sl[j][:, k2,
                                                 ct * 128:ct * 128 + ctw],
                                         w2c[:, k2, :],
                                         start=(k2 == 0), stop=(k2 == MT - 1))
                    gbc = gat_sb[:, j, ct:ct + 1].to_broadcast([128, NW])
                    nc.vector.tensor_tensor(yb[:, ct, :NW], p4a[:], gbc,
                                            OP.mult)
                    nc.vector.tensor_tensor(yb[:, ct, NW:], p4b[:], gbc,
                                            OP.mult)
                nc.gpsimd.dma_scatter_add(
                    y_p[p][:], yb[:], idxs_sb[:, j, :capj // 16],
                    capj, capj, PW)

            def zfill(p, half):
                blocks = range(half * ((ZBLK + 1) // 2),
                               min(ZBLK, (half + 1) * ((ZBLK + 1) // 2)))
                for bi, b in enumerate(blocks):
                    eng = nc.sync if bi % 2 == 0 else nc.scalar
                    eng.dma_start(y_p[p][b * 128:(b + 1) * 128, :NW], ztile[:])
                    eng2 = nc.scalar if bi % 2 == 0 else nc.sync
                    eng2.dma_start(y_p[p][b * 128:(b + 1) * 128, NW:], ztile[:])

            # pair-0 accumulator zeroed up-front (its scatters start mid-A)
            zfill(0, 0)
            zfill(0, 1)

            # ---------------- phase A: gathers + up-proj -> g; pair-0 -------
            # down-proj + scatter interleaved per expert so RS(0) can start
            # right at the end of A.
            for j in range(EPC):
                xgt_c = []
                c0 = 0
                for ci, cw in enumerate(CHUNKS[j]):
                    xgt = xgtp.tile([128, KT, cw], FP16, tag=f"xg{ci}",
                                    name=f"xg{ci}")
                    nc.gpsimd.dma_gather(
                        xgt[:], x16[:],
                        idx_sb[:, j, c0 // 16:(c0 + cw) // 16],
                        cw, cw, H, transpose=True)
                    xgt_c.append(xgt)
                    c0 += cw
                for m in range(MT):
                    w13t = wstream.tile([128, KT, 256], FP16, tag="w")
                    nc.sync.dma_start(
                        w13t[:], w13[j, m].rearrange("p (k c) -> p k c", c=256))
                    c0 = 0
                    for ci, cw in enumerate(CHUNKS[j]):
                        # compute width trimmed to the 16-granular capacity;
                        # the g tail [cwc, cw) stays garbage -> zero gate ->
                        # scatters to the dummy row.
                        cwc = min(cw, CAPS16[j] - c0)
                        p1 = psum.tile([128, 512], F32, tag="p1")
                        p3 = psum.tile([128, 512], F32, tag="p3")
                        for k in range(KT):
                            nc.tensor.matmul(p1[:, :cwc], w13t[:, k, :128],
                                             xgt_c[ci][:, k, :cwc],
                                             start=(k == 0), stop=(k == KT - 1))
                        for k in range(KT):
                            nc.tensor.matmul(p3[:, :cwc], w13t[:, k, 128:],
                                             xgt_c[ci][:, k, :cwc],
                                             start=(k == 0), stop=(k == KT - 1))
                        nc.scalar.activation(g_sl[j][:, m, c0:c0 + cwc],
                                             p1[:, :cwc], act)
                        nc.vector.tensor_tensor(g_sl[j][:, m, c0:c0 + cwc],
                                                g_sl[j][:, m, c0:c0 + cwc],
                                                p3[:, :cwc], OP.mult)
                        c0 += cw
                # pair-0 down-proj for this expert, then pair-1 zero-fill
                bcol(0, j)
                if j < NP:
                    zfill(1, j)

            nc.gpsimd.collective_compute(
                "ReduceScatter", OP.add,
                replica_groups=[list(range(NCORES))],
                ins=[y_p[0][0:T, :]],
                outs=[rs_p[0][:]],
            )

            # ---------------- phase B: pair-1 down-proj + RS ----------------
            for j in range(EPC):
                bcol(1, j)
            nc.gpsimd.collective_compute(
                "ReduceScatter", OP.add,
                replica_groups=[list(range(NCORES))],
                ins=[y_p[1][0:T, :]],
                outs=[rs_p[1][:]],
            )

            # ---------------- phase S: rest of shared intermediate ----------
            for sm in range(S1A, SIT):
                shared_int(sm)

            # ---------------- phase C: shared out + combine with rs ---------
            for n in range(H // NW):
                sh = []
                for hh in range(2):
                    s2 = wstream.tile([128, SIT // 2, NW], FP16, tag="w")
                    nc.scalar.dma_start(
                        s2[:], sw2b[n].rearrange("p (k c) -> p k c", c=NW)
                        [:, hh * (SIT // 2):(hh + 1) * (SIT // 2), :])
                    sh.append(s2)
                for ts in range(TS):
                    po = psum.tile([128, NW], F32, tag="p4a")
                    for k2 in range(SIT):
                        nc.tensor.matmul(po[:], gs[:, k2, ts * 128:(ts + 1) * 128],
                                         sh[k2 // (SIT // 2)][:, k2 % (SIT // 2), :],
                                         start=(k2 == 0), stop=(k2 == SIT - 1))
                    rst = small.tile([128, NW], FP16, tag="rst")
                    nc.sync.dma_start(
                        rst[:],
                        rs_p[n // 2][ts * 128:(ts + 1) * 128,
                                     (n % 2) * NW:(n % 2 + 1) * NW])
                    ott = small.tile([128, NW], FP16, tag="ott")
                    nc.vector.tensor_tensor(ott[:], po[:], rst[:], OP.add)
                    nc.sync.dma_start(
                        out[ts * 128:(ts + 1) * 128, n * NW:(n + 1) * NW], ott[:])

    if compile_:
        nc.compile()
    else:
        nc.insert_library_loads()
    return nc


def host_prep(hidden_states, gate_weight, w1, w2, w3, sw1, sw2, sw3):
    B, S, H = hidden_states.shape
    T = B * S
    E, I = w1.shape[0], w1.shape[1]
    SI = sw1.shape[0]
    EPC = E // NCORES
    KT, MT, SIT = H // 128, I // 128, SI // 128
    N4 = max(H // 512, 1)
    NW = min(H, 512)
    TOUT = T // NCORES

    x = np.ascontiguousarray(hidden_states.reshape(T, H), dtype=np.float32)
    combine = _routing(x, gate_weight.astype(np.float32))
    tok_lists = [np.nonzero(combine[:, e])[0] for e in range(E)]
    counts = np.array([len(t) for t in tok_lists])

    # balance experts over cores: serpentine over count-sorted experts; slot s
    # on every core holds its s-th largest expert, so per-slot capacity is the
    # max over cores of that rank band.
    order = np.argsort(-counts, kind="stable")
    assign = np.zeros((NCORES, EPC), np.int64)
    for s in range(EPC):
        band = order[s * NCORES:(s + 1) * NCORES]
        if s % 2 == 1:
            band = band[::-1]
        for c in range(NCORES):
            assign[c, s] = band[c]
    CAPS = tuple(
        int(max(128, ((counts[assign[:, s]].max() + 127) // 128) * 128))
        for s in range(EPC))
    CAPS16 = tuple(
        int(max(128, ((counts[assign[:, s]].max() + 15) // 16) * 16))
        for s in range(EPC))
    CAP0 = max(CAPS)
    CT0 = CAP0 // 128

    x16 = x.astype(np.float16)
    xT = x.T  # [H, T] view

    s1 = sw1.T.reshape(KT, 128, SIT, 128).transpose(2, 1, 0, 3)
    s3 = sw3.T.reshape(KT, 128, SIT, 128).transpose(2, 1, 0, 3)
    sw13 = np.ascontiguousarray(
        np.concatenate([s1, s3], axis=-1).reshape(SIT, 128, -1), dtype=np.float16)
    sw2b = np.ascontiguousarray(
        sw2.T.reshape(SIT, 128, N4, NW).transpose(2, 1, 0, 3).reshape(N4, 128, -1),
        dtype=np.float16)

    in_maps = []
    for c in range(NCORES):
        els = list(assign[c])
        idx_np = np.zeros((EPC, 128, CAP0 // 16), np.int16)
        idxs_np = np.zeros((EPC, 128, CAP0 // 16), np.int16)
        gat_np = np.zeros((EPC, 128, CT0), np.float32)
        for j, e in enumerate(els):
            cap = CAPS[j]
            toks = tok_lists[e]
            a = np.zeros(cap, np.int16)
            a[:len(toks)] = toks
            idx_np[j, :, :cap // 16] = np.tile(a.reshape(cap // 16, 16).T, (8, 1))
            b2 = np.full(cap, T, np.int16)  # pad rows land on dummy row T
            b2[:len(toks)] = toks
            idxs_np[j, :, :cap // 16] = np.tile(b2.reshape(cap // 16, 16).T, (8, 1))
            gv = np.zeros(cap, np.float32)
            gv[:len(toks)] = combine[toks, e]
            gat_np[j, :, :cap // 128] = gv.reshape(cap // 128, 128).T
        w13c = np.empty((EPC, MT, 128, KT * 256), np.float16)
        w2c = np.empty((EPC, N4, 128, MT * NW), np.float16)
        for j, e in enumerate(els):
            a1 = w1[e].T.reshape(KT, 128, MT, 128).transpose(2, 1, 0, 3)
            a3 = w3[e].T.reshape(KT, 128, MT, 128).transpose(2, 1, 0, 3)
            w13c[j] = np.concatenate([a1, a3], axis=-1).reshape(MT, 128, -1)
            w2c[j] = (w2[e].T.reshape(MT, 128, N4, NW)
                      .transpose(2, 1, 0, 3).reshape(N4, 128, -1))
        xTc = np.ascontiguousarray(
            xT[:, c * TOUT:(c + 1) * TOUT].reshape(KT, 128, TOUT)
            .transpose(1, 0, 2).reshape(128, -1), dtype=np.float16)
        in_maps.append({
            "x16": x16, "xTc": xTc,
            "w13": w13c, "w2b": w2c,
            "sw13": sw13, "sw2b": sw2b,
            "idx": idx_np, "idxs": idxs_np, "gat": gat_np,
        })
    cfg = dict(T=T, H=H, I=I, CAPS=CAPS, CAPS16=CAPS16, SI=SI)
    return in_maps, cfg


def kernel(**inputs):
    inputs = {k: np.asarray(v) for k, v in inputs.items()}
    hs = inputs["hidden_states"]
    B, S, H = hs.shape
    in_maps, cfg = host_prep(
        hs, inputs["gate_weight"], inputs["w1"], inputs["w2"], inputs["w3"],
        inputs["sw1"], inputs["sw2"], inputs["sw3"])
    nc = build_kernel(**cfg)
    res = run_bass_kernel_spmd(nc, in_maps, list(range(NCORES)))
    y = np.concatenate([res.results[c]["out"] for c in range(NCORES)], axis=0)
    return y.reshape(B, S, H).astype(np.float32)


if __name__ == "__main__":
    pass


# revision 14
# speedup vs baseline: 1.0299x; 1.0029x over previous
"""DeepSeekV2 MoE layer on 8 trn2 NeuronCores (expert-parallel).

Strategy (v5):
  - Host: gate softmax + group-limited top-k routing -> per-expert token index
    lists and combine weights (control data only; all heavy FLOPs on device).
    Experts are load-balanced across cores (serpentine over counts) and each
    core's 4 expert slots get per-slot capacities (max over cores, ceil 128).
  - Device (SPMD over 8 cores, 4 expert slots each):
      A: per slot: transposed fp16 dma_gather (double-buffered, spread over
         4 SWDGE queues) -> mm1/mm3 fp16 -> silu*mul -> g[slot] in SBUF;
         zero-fill of the y accumulators is interleaved here (sync/scalar);
      B: column-PAIR major down-proj: for each 1024-wide column pair: all 4
         slots' mm2 + gate-scale + one 2KB-elem dma_scatter_add per slot
         (negative-index padding skips pad tokens), then ReduceScatter(add)
         for the pair -> the 2 RS's overlap the shared-expert phase;
      S: shared-expert intermediate for own 512-token slice (after B so the
         RS chain hides under it; first S1A iters run up-front as warmup
         filler while the first gather lands);
      C: shared out matmuls + add RS result -> out.
  - Host: concatenate 512-row slices -> [B, S, H].
"""
import sys

import numpy as np

sys.path.insert(0, "/opt/trn_rl_repo")

import concourse.bass as bass
import concourse.mybir as mybir
import concourse.tile as tile
from concourse import bacc
from concourse.bass_utils import run_bass_kernel_spmd

F32 = mybir.dt.float32
FP16 = mybir.dt.float16
I16 = mybir.dt.int16
AF = mybir.ActivationFunctionType
OP = mybir.AluOpType

N_GROUP, TOPK_GROUP, TOP_K = 8, 3, 6
NCORES = 8
S1A = 6  # shared-intermediate iters run before phase A (warmup filler)


def _routing(x, gate_w):
    T, E = x.shape[0], gate_w.shape[0]
    logits = (x @ gate_w.T).astype(np.float64)
    e = np.exp(logits - logits.max(-1, keepdims=True))
    scores = e / e.sum(-1, keepdims=True)
    per_group = E // N_GROUP
    group_scores = scores.reshape(T, N_GROUP, per_group).max(-1)
    order = np.argsort(-group_scores, axis=-1, kind="stable")
    group_mask = np.zeros((T, N_GROUP), bool)
    np.put_along_axis(group_mask, order[:, :TOPK_GROUP], True, axis=1)
    tmp = np.where(np.repeat(group_mask, per_group, axis=1), scores, 0.0)
    order_e = np.argsort(-tmp, axis=-1, kind="stable")
    topk_idx = order_e[:, :TOP_K]
    topk_w = np.take_along_axis(tmp, topk_idx, axis=1)
    topk_w = topk_w / (topk_w.sum(-1, keepdims=True) + 1e-20)
    combine = np.zeros((T, E), np.float32)
    np.put_along_axis(combine, topk_idx, topk_w.astype(np.float32), axis=1)
    return combine


def _chunks(cap):
    out, rem = [], cap
    while rem:
        if rem <= 512:
            out.append(rem)
            rem = 0
        elif rem == 640:
            out.append(384)
            rem = 256
        else:
            out.append(512)
            rem -= 512
    return out


def build_kernel(T, H, I, CAPS, CAPS16, SI, act=AF.Silu, compile_=True):
    EPC = len(CAPS)
    KT = H // 128         # H contraction tiles
    MT = I // 128         # I tiles
    NP = max(H // 1024, 1)  # column pairs
    PW = min(H, 1024)       # pair width
    NW = 512
    SIT = SI // 128       # shared-intermediate tiles
    TOUT = T // NCORES    # own token slice
    TS = TOUT // 128
    CAP0 = max(CAPS)
    CT0 = CAP0 // 128
    CHUNKS = [_chunks(c) for c in CAPS]
    ZBLK = (T + 128) // 128  # zero blocks per pair tensor

    nc = bacc.Bacc("TRN2")
    x16 = nc.dram_tensor("x16", [T, H], FP16, kind="ExternalInput")
    xTc = nc.dram_tensor("xTc", [128, KT * TOUT], FP16, kind="ExternalInput")
    w13 = nc.dram_tensor("w13", [EPC, MT, 128, KT * 256], FP16, kind="ExternalInput")
    w2b = nc.dram_tensor("w2b", [EPC, H // NW, 128, MT * NW], FP16,
                         kind="ExternalInput")
    sw13 = nc.dram_tensor("sw13", [SIT, 128, KT * 256], FP16, kind="ExternalInput")
    sw2b = nc.dram_tensor("sw2b", [H // NW, 128, SIT * NW], FP16,
                          kind="ExternalInput")
    idx = nc.dram_tensor("idx", [EPC, 128, CAP0 // 16], I16, kind="ExternalInput")
    idxs = nc.dram_tensor("idxs", [EPC, 128, CAP0 // 16], I16, kind="ExternalInput")
    gat = nc.dram_tensor("gat", [EPC, 128, CT0], F32, kind="ExternalInput")
    out = nc.dram_tensor("out", [TOUT, H], FP16, kind="ExternalOutput")

    y_p = [nc.dram_tensor(f"y_pair{p}", [T + 128, PW], FP16) for p in range(NP)]
    rs_p = [nc.dram_tensor(f"rs_pair{p}", [TOUT, PW], FP16) for p in range(NP)]

    with tile.TileContext(nc) as tc:
        with (
            tc.tile_pool(name="const", bufs=1) as const,
            tc.tile_pool(name="persist", bufs=1) as persist,
            tc.tile_pool(name="xgtp", bufs=2) as xgtp,
            tc.tile_pool(name="wstream", bufs=4) as wstream,
            tc.tile_pool(name="ybp", bufs=2) as ybp,
            tc.tile_pool(name="small", bufs=2) as small,
            tc.tile_pool(name="psum", bufs=2, space="PSUM") as psum,
        ):
            idx_sb = const.tile([128, EPC, CAP0 // 16], I16)
            nc.scalar.dma_start(idx_sb[:], idx.rearrange("e p c -> p e c"))
            idxs_sb = const.tile([128, EPC, CAP0 // 16], I16)
            nc.scalar.dma_start(idxs_sb[:], idxs.rearrange("e p c -> p e c"))
            gat_sb = const.tile([128, EPC, CT0], F32)
            nc.scalar.dma_start(gat_sb[:], gat.rearrange("e p c -> p e c"))
            # shared-expert input (own tokens, H-tiled on partitions)
            xtc_sb = persist.tile([128, KT, TOUT], FP16)
            nc.scalar.dma_start(xtc_sb[:], xTc[:])
            gs = persist.tile([128, SIT, TOUT], FP16)
            g_sl = [persist.tile([128, MT, CAPS16[j]], FP16, tag=f"g{j}",
                                 name=f"g{j}")
                    for j in range(EPC)]
            ztile = const.tile([128, NW], FP16)
            nc.vector.memset(ztile[:], 0.0)

            def shared_int(sm):
                # warmup iters stay on xg1 (xg0 must stay free for the first
                # gather); post-B iters alternate both tags and both HWDGE
                # rings for 4-deep prefetch under RS-induced HBM contention.
                tag = "xg1" if sm < S1A else f"xg{sm % 2}"
                eng = nc.sync if sm % 2 == 0 else nc.scalar
                s13 = xgtp.tile([128, KT, 256], FP16, tag=tag, name="s13")
                eng.dma_start(
                    s13[:], sw13[sm].rearrange("p (k c) -> p k c", c=256))
                p1 = psum.tile([128, 512], F32, tag="p1")
                p3 = psum.tile([128, 512], F32, tag="p3")
                for k in range(KT):
                    nc.tensor.matmul(p1[:, :TOUT], s13[:, k, :128], xtc_sb[:, k, :],
                                     start=(k == 0), stop=(k == KT - 1))
                for k in range(KT):
                    nc.tensor.matmul(p3[:, :TOUT], s13[:, k, 128:], xtc_sb[:, k, :],
                                     start=(k == 0), stop=(k == KT - 1))
                nc.scalar.activation(gs[:, sm, :], p1[:, :TOUT], act)
                nc.vector.tensor_tensor(gs[:, sm, :], gs[:, sm, :], p3[:, :TOUT],
                                        OP.mult)

            # warmup filler while the first gathers land
            for sm in range(S1A):
                shared_int(sm)

            def bcol(p, j):
                capj = CAPS[j]
                ctj = capj // 128
                w2a = wstream.tile([128, MT, NW], FP16, tag="w", name="w2a")
                nc.scalar.dma_start(
                    w2a[:], w2b[j, 2 * p].rearrange("p (k c) -> p k c", c=NW))
                w2c = wstream.tile([128, MT, NW], FP16, tag="w", name="w2c")
                nc.scalar.dma_start(
                    w2c[:], w2b[j, 2 * p + 1].rearrange("p (k c) -> p k c",
                                                        c=NW))
                yb = ybp.tile([128, ctj, PW], FP16, tag="yb", name="yb")
                for ct in range(ctj):
                    ctw = min(128, CAPS16[j] - ct * 128)
                    p4a = psum.tile([128, NW], F32, tag="p4a")
                    p4b = psum.tile([128, NW], F32, tag="p4b")
                    for k2 in range(MT):
                        nc.tensor.matmul(p4a[:ctw],
                                         g_sl[j][:, k2,
                                                 ct * 128:ct * 128 + ctw],
                                         w2a[:, k2, :],
                                         start=(k2 == 0), stop=(k2 == MT - 1))
                    for k2 in range(MT):
                        nc.tensor.matmul(p4b[:ctw],
                                         g_sl[j][:, k2,
                                                 ct * 128:ct * 128 + ctw],
                                         w2c[:, k2, :],
                                         start=(k2 == 0), stop=(k2 == MT - 1))
                    gbc = gat_sb[:, j, ct:ct + 1].to_broadcast([128, NW])
                    nc.vector.tensor_tensor(yb[:, ct, :NW], p4a[:], gbc,
                                            OP.mult)
                    nc.vector.tensor_tensor(yb[:, ct, NW:], p4b[:], gbc,
                                            OP.mult)
                nc.gpsimd.dma_scatter_add(
                    y_p[p][:], yb[:], idxs_sb[:, j, :capj // 16],
                    capj, capj, PW)

            def zfill(p, half):
                blocks = range(half * ((ZBLK + 1) // 2),
                               min(ZBLK, (half + 1) * ((ZBLK + 1) // 2)))
                for bi, b in enumerate(blocks):
                    eng = nc.sync if bi % 2 == 0 else nc.scalar
                    eng.dma_start(y_p[p][b * 128:(b + 1) * 128, :NW], ztile[:])
                    eng2 = nc.scalar if bi % 2 == 0 else nc.sync
                    eng2.dma_start(y_p[p][b * 128:(b + 1) * 128, NW:], ztile[:])

            # pair-0 accumulator zeroed up-front (its scatters start mid-A)
            zfill(0, 0)
            zfill(0, 1)

            # ---------------- phase A: gathers + up-proj -> g; pair-0 -------
            # down-proj + scatter interleaved per expert so RS(0) can start
            # right at the end of A.
            for j in range(EPC):
                xgt_c = []
                c0 = 0
                for ci, cw in enumerate(CHUNKS[j]):
                    xgt = xgtp.tile([128, KT, cw], FP16, tag=f"xg{ci}",
                                    name=f"xg{ci}")
                    nc.gpsimd.dma_gather(
                        xgt[:], x16[:],
                        idx_sb[:, j, c0 // 16:(c0 + cw) // 16],
                        cw, cw, H, transpose=True)
                    xgt_c.append(xgt)
                    c0 += cw
                for m in range(MT):
                    w13t = wstream.tile([128, KT, 256], FP16, tag="w")
                    nc.sync.dma_start(
                        w13t[:], w13[j, m].rearrange("p (k c) -> p k c", c=256))
                    c0 = 0
                    for ci, cw in enumerate(CHUNKS[j]):
                        # compute width trimmed to the 16-granular capacity;
                        # the g tail [cwc, cw) stays garbage -> zero gate ->
                        # scatters to the dummy row.
                        cwc = min(cw, CAPS16[j] - c0)
                        p1 = psum.tile([128, 512], F32, tag="p1")
                        p3 = psum.tile([128, 512], F32, tag="p3")
                        for k in range(KT):
                            nc.tensor.matmul(p1[:, :cwc], w13t[:, k, :128],
                                             xgt_c[ci][:, k, :cwc],
                                             start=(k == 0), stop=(k == KT - 1))
                        for k in range(KT):
                            nc.tensor.matmul(p3[:, :cwc], w13t[:, k, 128:],
                                             xgt_c[ci][:, k, :cwc],
                                             start=(k == 0), stop=(k == KT - 1))
                        nc.scalar.activation(g_sl[j][:, m, c0:c0 + cwc],
                                             p1[:, :cwc], act)
                        nc.vector.tensor_tensor(g_sl[j][:, m, c0:c0 + cwc],
                                                g_sl[j][:, m, c0:c0 + cwc],
                                                p3[:, :cwc], OP.mult)
                        c0 += cw
                # pair-0 down-proj for this expert, then pair-1 zero-fill
                bcol(0, j)
                if j < NP:
                    zfill(1, j)

            nc.gpsimd.collective_compute(
                "ReduceScatter", OP.add,
                replica_groups=[list(range(NCORES))],
                ins=[y_p[0][0:T, :]],
                outs=[rs_p[0][:]],
            )

            # ---------------- phase B: pair-1 down-proj + RS ----------------
            for j in range(EPC):
                bcol(1, j)
            nc.gpsimd.collective_compute(
                "ReduceScatter", OP.add,
                replica_groups=[list(range(NCORES))],
                ins=[y_p[1][0:T, :]],
                outs=[rs_p[1][:]],
            )

            # ---------------- phase S: rest of shared intermediate ----------
            for sm in range(S1A, SIT):
                shared_int(sm)

            # ---------------- phase C: shared out + combine with rs ---------
            for n in range(H // NW):
                sh = []
                for hh in range(2):
                    s2 = wstream.tile([128, SIT // 2, NW], FP16, tag="w")
                    nc.scalar.dma_start(
                        s2[:], sw2b[n].rearrange("p (k c) -> p k c", c=NW)
                        [:, hh * (SIT // 2):(hh + 1) * (SIT // 2), :])
                    sh.append(s2)
                for ts in range(TS):
                    po = psum.tile([128, NW], F32, tag="p4a")
                    for k2 in range(SIT):
                        nc.tensor.matmul(po[:], gs[:, k2, ts * 128:(ts + 1) * 128],
                                         sh[k2 // (SIT // 2)][:, k2 % (SIT // 2), :],
                                         start=(k2 == 0), stop=(k2 == SIT - 1))
                    rst = small.tile([128, NW], FP16, tag="rst")
                    nc.sync.dma_start(
                        rst[:],
                        rs_p[n // 2][ts * 128:(ts + 1) * 128,
                                     (n % 2) * NW:(n % 2 + 1) * NW])
                    ott = small.tile([128, NW], FP16, tag="ott")
                    nc.vector.tensor_tensor(ott[:], po[:], rst[:], OP.add)
                    nc.sync.dma_start(
                        out[ts * 128:(ts + 1) * 128, n * NW:(n + 1) * NW], ott[:])

    if compile_:
        nc.compile()
    else:
        nc.insert_library_loads()
    return nc


def host_prep(hidden_states, gate_weight, w1, w2, w3, sw1, sw2, sw3):
    B, S, H = hidden_states.shape
    T = B * S
    E, I = w1.shape[0], w1.shape[1]
    SI = sw1.shape[0]
    EPC = E // NCORES
    KT, MT, SIT = H // 128, I // 128, SI // 128
    N4 = max(H // 512, 1)
    NW = min(H, 512)
    TOUT = T // NCORES

    x = np.ascontiguousarray(hidden_states.reshape(T, H), dtype=np.float32)
    combine = _routing(x, gate_weight.astype(np.float32))
    tok_lists = [np.nonzero(combine[:, e])[0] for e in range(E)]
    counts = np.array([len(t) for t in tok_lists])

    # balance experts over cores: serpentine over count-sorted experts; slot s
    # on every core holds its s-th largest expert, so per-slot capacity is the
    # max over cores of that rank band.
    order = np.argsort(-counts, kind="stable")
    assign = np.zeros((NCORES, EPC), np.int64)
    for s in range(EPC):
        band = order[s * NCORES:(s + 1) * NCORES]
        if s % 2 == 1:
            band = band[::-1]
        for c in range(NCORES):
            assign[c, s] = band[c]
    CAPS = tuple(
        int(max(128, ((counts[assign[:, s]].max() + 127) // 128) * 128))
        for s in range(EPC))
    CAPS16 = tuple(
        int(max(128, ((counts[assign[:, s]].max() + 15) // 16) * 16))
        for s in range(EPC))
    CAP0 = max(CAPS)
    CT0 = CAP0 // 128

    x16 = x.astype(np.float16)
    xT = x.T  # [H, T] view

    s1 = sw1.T.reshape(KT, 128, SIT, 128).transpose(2, 1, 0, 3)
    s3 = sw3.T.reshape(KT, 128, SIT, 128).transpose(2, 1, 0, 3)
    sw13 = np.ascontiguousarray(
        np.concatenate([s1, s3], axis=-1).reshape(SIT, 128, -1), dtype=np.float16)
    sw2b = np.ascontiguousarray(
        sw2.T.reshape(SIT, 128, N4, NW).transpose(2, 1, 0, 3).reshape(N4, 128, -1),
        dtype=np.float16)

    in_maps = []
    for c in range(NCORES):
        els = list(assign[c])
        idx_np = np.zeros((EPC, 128, CAP0 // 16), np.int16)
        idxs_np = np.zeros((EPC, 128, CAP0 // 16), np.int16)
        gat_np = np.zeros((EPC, 128, CT0), np.float32)
        for j, e in enumerate(els):
            cap = CAPS[j]
            toks = tok_lists[e]
            a = np.zeros(cap, np.int16)
            a[:len(toks)] = toks
            idx_np[j, :, :cap // 16] = np.tile(a.reshape(cap // 16, 16).T, (8, 1))
            b2 = np.full(cap, T, np.int16)  # pad rows land on dummy row T
            b2[:len(toks)] = toks
            idxs_np[j, :, :cap // 16] = np.tile(b2.reshape(cap // 16, 16).T, (8, 1))
            gv = np.zeros(cap, np.float32)
            gv[:len(toks)] = combine[toks, e]
            gat_np[j, :, :cap // 128] = gv.reshape(cap // 128, 128).T
        w13c = np.empty((EPC, MT, 128, KT * 256), np.float16)
        w2c = np.empty((EPC, N4, 128, MT * NW), np.float16)
        for j, e in enumerate(els):
            a1 = w1[e].T.reshape(KT, 128, MT, 128).transpose(2, 1, 0, 3)
            a3 = w3[e].T.reshape(KT, 128, MT, 128).transpose(2, 1, 0, 3)
            w13c[j] = np.concatenate([a1, a3], axis=-1).reshape(MT, 128, -1)
            w2c[j] = (w2[e].T.reshape(MT, 128, N4, NW)
                      .transpose(2, 1, 0, 3).reshape(N4, 128, -1))
        xTc = np.ascontiguousarray(
            xT[:, c * TOUT:(c + 1) * TOUT].reshape(KT, 128, TOUT)
            .transpose(1, 0, 2).reshape(128, -1), dtype=np.float16)
        in_maps.append({
            "x16": x16, "xTc": xTc,
            "w13": w13c, "w2b": w2c,
            "sw13": sw13, "sw2b": sw2b,
            "idx": idx_np, "idxs": idxs_np, "gat": gat_np,
        })
    cfg = dict(T=T, H=H, I=I, CAPS=CAPS, CAPS16=CAPS16, SI=SI)
    return in_maps, cfg


def kernel(**inputs):
    inputs = {k: np.asarray(v) for k, v in inputs.items()}
    hs = inputs["hidden_states"]
    B, S, H = hs.shape
    in_maps, cfg = host_prep(
        hs, inputs["gate_weight"], inputs["w1"], inputs["w2"], inputs["w3"],
        inputs["sw1"], inputs["sw2"], inputs["sw3"])
    nc = build_kernel(**cfg)
    res = run_bass_kernel_spmd(nc, in_maps, list(range(NCORES)))
    y = np.concatenate([res.results[c]["out"] for c in range(NCORES)], axis=0)
    return y.reshape(B, S, H).astype(np.float32)


if __name__ == "__main__":
    pass


# revision 17
# speedup vs baseline: 1.0370x; 1.0069x over previous
"""DeepSeekV2 MoE layer on 8 trn2 NeuronCores (expert-parallel).

Strategy (v5):
  - Host: gate softmax + group-limited top-k routing -> per-expert token index
    lists and combine weights (control data only; all heavy FLOPs on device).
    Experts are load-balanced across cores (serpentine over counts) and each
    core's 4 expert slots get per-slot capacities (max over cores, ceil 128).
  - Device (SPMD over 8 cores, 4 expert slots each):
      A: per slot: transposed fp16 dma_gather (double-buffered, spread over
         4 SWDGE queues) -> mm1/mm3 fp16 -> silu*mul -> g[slot] in SBUF;
         zero-fill of the y accumulators is interleaved here (sync/scalar);
      B: column-PAIR major down-proj: for each 1024-wide column pair: all 4
         slots' mm2 + gate-scale + one 2KB-elem dma_scatter_add per slot
         (negative-index padding skips pad tokens), then ReduceScatter(add)
         for the pair -> the 2 RS's overlap the shared-expert phase;
      S: shared-expert intermediate for own 512-token slice (after B so the
         RS chain hides under it; first S1A iters run up-front as warmup
         filler while the first gather lands);
      C: shared out matmuls + add RS result -> out.
  - Host: concatenate 512-row slices -> [B, S, H].
"""
import sys

import numpy as np

sys.path.insert(0, "/opt/trn_rl_repo")

import concourse.bass as bass
import concourse.mybir as mybir
import concourse.tile as tile
from concourse import bacc
from concourse.bass_utils import run_bass_kernel_spmd

F32 = mybir.dt.float32
FP16 = mybir.dt.float16
I16 = mybir.dt.int16
AF = mybir.ActivationFunctionType
OP = mybir.AluOpType

N_GROUP, TOPK_GROUP, TOP_K = 8, 3, 6
NCORES = 8
S1A = 6  # shared-intermediate iters run before phase A (warmup filler)


def _routing(x, gate_w):
    T, E = x.shape[0], gate_w.shape[0]
    logits = (x @ gate_w.T).astype(np.float64)
    e = np.exp(logits - logits.max(-1, keepdims=True))
    scores = e / e.sum(-1, keepdims=True)
    per_group = E // N_GROUP
    group_scores = scores.reshape(T, N_GROUP, per_group).max(-1)
    order = np.argsort(-group_scores, axis=-1, kind="stable")
    group_mask = np.zeros((T, N_GROUP), bool)
    np.put_along_axis(group_mask, order[:, :TOPK_GROUP], True, axis=1)
    tmp = np.where(np.repeat(group_mask, per_group, axis=1), scores, 0.0)
    order_e = np.argsort(-tmp, axis=-1, kind="stable")
    topk_idx = order_e[:, :TOP_K]
    topk_w = np.take_along_axis(tmp, topk_idx, axis=1)
    topk_w = topk_w / (topk_w.sum(-1, keepdims=True) + 1e-20)
    combine = np.zeros((T, E), np.float32)
    np.put_along_axis(combine, topk_idx, topk_w.astype(np.float32), axis=1)
    return combine


def _chunks(cap):
    out, rem = [], cap
    while rem:
        if rem <= 512:
            out.append(rem)
            rem = 0
        elif rem == 640:
            out.append(384)
            rem = 256
        else:
            out.append(512)
            rem -= 512
    return out


def build_kernel(T, H, I, CAPS, CAPS16, SI, act=AF.Silu, compile_=True):
    EPC = len(CAPS)
    KT = H // 128         # H contraction tiles
    MT = I // 128         # I tiles
    NP = max(H // 1024, 1)  # column pairs
    PW = min(H, 1024)       # pair width
    NW = 512
    SIT = SI // 128       # shared-intermediate tiles
    TOUT = T // NCORES    # own token slice
    TS = TOUT // 128
    CAP0 = max(CAPS)
    CT0 = CAP0 // 128
    CHUNKS = [_chunks(c) for c in CAPS]
    ZBLK = (T + 128) // 128  # zero blocks per pair tensor

    nc = bacc.Bacc("TRN2")
    x16 = nc.dram_tensor("x16", [T, H], FP16, kind="ExternalInput")
    xTc = nc.dram_tensor("xTc", [128, KT * TOUT], FP16, kind="ExternalInput")
    w13 = nc.dram_tensor("w13", [EPC, MT, 128, KT * 256], FP16, kind="ExternalInput")
    w2b = nc.dram_tensor("w2b", [EPC, H // NW, 128, MT * NW], FP16,
                         kind="ExternalInput")
    sw13 = nc.dram_tensor("sw13", [SIT, 128, KT * 256], FP16, kind="ExternalInput")
    sw2b = nc.dram_tensor("sw2b", [H // NW, 128, SIT * NW], FP16,
                          kind="ExternalInput")
    idx = nc.dram_tensor("idx", [EPC, 128, CAP0 // 16], I16, kind="ExternalInput")
    idxs = nc.dram_tensor("idxs", [EPC, 128, CAP0 // 16], I16, kind="ExternalInput")
    gat = nc.dram_tensor("gat", [EPC, 128, CT0], F32, kind="ExternalInput")
    out = nc.dram_tensor("out", [TOUT, H], FP16, kind="ExternalOutput")

    y_p = [nc.dram_tensor(f"y_pair{p}", [T + 128, PW], FP16) for p in range(NP)]
    rs_p = [nc.dram_tensor(f"rs_pair{p}", [TOUT, PW], FP16) for p in range(NP)]

    with tile.TileContext(nc) as tc:
        with (
            tc.tile_pool(name="const", bufs=1) as const,
            tc.tile_pool(name="persist", bufs=1) as persist,
            tc.tile_pool(name="xgtp", bufs=2) as xgtp,
            tc.tile_pool(name="wstream", bufs=4) as wstream,
            tc.tile_pool(name="ybp", bufs=2) as ybp,
            tc.tile_pool(name="small", bufs=2) as small,
            tc.tile_pool(name="psum", bufs=2, space="PSUM") as psum,
        ):
            idx_sb = const.tile([128, EPC, CAP0 // 16], I16)
            nc.scalar.dma_start(idx_sb[:], idx.rearrange("e p c -> p e c"))
            idxs_sb = const.tile([128, EPC, CAP0 // 16], I16)
            nc.scalar.dma_start(idxs_sb[:], idxs.rearrange("e p c -> p e c"))
            gat_sb = const.tile([128, EPC, CT0], F32)
            nc.scalar.dma_start(gat_sb[:], gat.rearrange("e p c -> p e c"))
            # shared-expert input (own tokens, H-tiled on partitions)
            xtc_sb = persist.tile([128, KT, TOUT], FP16)
            nc.scalar.dma_start(xtc_sb[:], xTc[:])
            gs = persist.tile([128, SIT, TOUT], FP16)
            g_sl = [persist.tile([128, MT, CAPS16[j]], FP16, tag=f"g{j}",
                                 name=f"g{j}")
                    for j in range(EPC)]
            ztile = const.tile([128, NW], FP16)
            nc.vector.memset(ztile[:], 0.0)

            def shared_int(sm):
                # warmup iters stay on xg1 (xg0 must stay free for the first
                # gather); post-B iters alternate both tags and both HWDGE
                # rings for 4-deep prefetch under RS-induced HBM contention.
                tag = "xg1" if sm < S1A else f"xg{sm % 2}"
                eng = nc.sync if sm % 2 == 0 else nc.scalar
                s13 = xgtp.tile([128, KT, 256], FP16, tag=tag, name="s13")
                eng.dma_start(
                    s13[:], sw13[sm].rearrange("p (k c) -> p k c", c=256))
                p1 = psum.tile([128, 512], F32, tag="p1")
                p3 = psum.tile([128, 512], F32, tag="p3")
                for k in range(KT):
                    nc.tensor.matmul(p1[:, :TOUT], s13[:, k, :128], xtc_sb[:, k, :],
                                     start=(k == 0), stop=(k == KT - 1))
                for k in range(KT):
                    nc.tensor.matmul(p3[:, :TOUT], s13[:, k, 128:], xtc_sb[:, k, :],
                                     start=(k == 0), stop=(k == KT - 1))
                nc.scalar.activation(gs[:, sm, :], p1[:, :TOUT], act)
                nc.vector.tensor_tensor(gs[:, sm, :], gs[:, sm, :], p3[:, :TOUT],
                                        OP.mult)

            # warmup filler while the first gathers land
            for sm in range(S1A):
                shared_int(sm)

            def bcol(p, j):
                capj = CAPS[j]
                ctj = capj // 128
                w2a = wstream.tile([128, MT, NW], FP16, tag="w", name="w2a")
                nc.scalar.dma_start(
                    w2a[:], w2b[j, 2 * p].rearrange("p (k c) -> p k c", c=NW))
                w2c = wstream.tile([128, MT, NW], FP16, tag="w", name="w2c")
                nc.scalar.dma_start(
                    w2c[:], w2b[j, 2 * p + 1].rearrange("p (k c) -> p k c",
                                                        c=NW))
                yb = ybp.tile([128, ctj, PW], FP16, tag="yb", name="yb")
                for ct in range(ctj):
                    ctw = min(128, CAPS16[j] - ct * 128)
                    p4a = psum.tile([128, NW], F32, tag="p4a")
                    p4b = psum.tile([128, NW], F32, tag="p4b")
                    for k2 in range(MT):
                        nc.tensor.matmul(p4a[:ctw],
                                         g_sl[j][:, k2,
                                                 ct * 128:ct * 128 + ctw],
                                         w2a[:, k2, :],
                                         start=(k2 == 0), stop=(k2 == MT - 1))
                    for k2 in range(MT):
                        nc.tensor.matmul(p4b[:ctw],
                                         g_sl[j][:, k2,
                                                 ct * 128:ct * 128 + ctw],
                                         w2c[:, k2, :],
                                         start=(k2 == 0), stop=(k2 == MT - 1))
                    gbc = gat_sb[:, j, ct:ct + 1].to_broadcast([128, NW])
                    nc.vector.tensor_tensor(yb[:, ct, :NW], p4a[:], gbc,
                                            OP.mult)
                    nc.vector.tensor_tensor(yb[:, ct, NW:], p4b[:], gbc,
                                            OP.mult)
                # split scatter: first half enqueues as soon as half the
                # yb tile is written, starting the SWDGE drain ~12us earlier
                cth = ctj // 2
                nc.gpsimd.dma_scatter_add(
                    y_p[p][:], yb[:, :cth, :], idxs_sb[:, j, :cth * 8],
                    cth * 128, cth * 128, PW)
                nc.gpsimd.dma_scatter_add(
                    y_p[p][:], yb[:, cth:, :],
                    idxs_sb[:, j, cth * 8:capj // 16],
                    capj - cth * 128, capj - cth * 128, PW)

            def zfill(p, half):
                blocks = range(half * ((ZBLK + 1) // 2),
                               min(ZBLK, (half + 1) * ((ZBLK + 1) // 2)))
                for bi, b in enumerate(blocks):
                    eng = nc.sync if bi % 2 == 0 else nc.scalar
                    eng.dma_start(y_p[p][b * 128:(b + 1) * 128, :NW], ztile[:])
                    eng2 = nc.scalar if bi % 2 == 0 else nc.sync
                    eng2.dma_start(y_p[p][b * 128:(b + 1) * 128, NW:], ztile[:])

            # pair-0 accumulator zeroed up-front (its scatters start mid-A)
            zfill(0, 0)
            zfill(0, 1)

            # ---------------- phase A: gathers + up-proj -> g; pair-0 -------
            # down-proj + scatter interleaved per expert so RS(0) can start
            # right at the end of A.
            for j in range(EPC):
                xgt_c = []
                c0 = 0
                for ci, cw in enumerate(CHUNKS[j]):
                    xgt = xgtp.tile([128, KT, cw], FP16, tag=f"xg{ci}",
                                    name=f"xg{ci}")
                    nc.gpsimd.dma_gather(
                        xgt[:], x16[:],
                        idx_sb[:, j, c0 // 16:(c0 + cw) // 16],
                        cw, cw, H, transpose=True)
                    xgt_c.append(xgt)
                    c0 += cw
                for m in range(MT):
                    w13t = wstream.tile([128, KT, 256], FP16, tag="w")
                    nc.sync.dma_start(
                        w13t[:], w13[j, m].rearrange("p (k c) -> p k c", c=256))
                    c0 = 0
                    for ci, cw in enumerate(CHUNKS[j]):
                        # compute width trimmed to the 16-granular capacity;
                        # the g tail [cwc, cw) stays garbage -> zero gate ->
                        # scatters to the dummy row.
                        cwc = min(cw, CAPS16[j] - c0)
                        p1 = psum.tile([128, 512], F32, tag="p1")
                        p3 = psum.tile([128, 512], F32, tag="p3")
                        for k in range(KT):
                            nc.tensor.matmul(p1[:, :cwc], w13t[:, k, :128],
                                             xgt_c[ci][:, k, :cwc],
                                             start=(k == 0), stop=(k == KT - 1))
                        for k in range(KT):
                            nc.tensor.matmul(p3[:, :cwc], w13t[:, k, 128:],
                                             xgt_c[ci][:, k, :cwc],
                                             start=(k == 0), stop=(k == KT - 1))
                        nc.scalar.activation(g_sl[j][:, m, c0:c0 + cwc],
                                             p1[:, :cwc], act)
                        nc.vector.tensor_tensor(g_sl[j][:, m, c0:c0 + cwc],
                                                g_sl[j][:, m, c0:c0 + cwc],
                                                p3[:, :cwc], OP.mult)
                        c0 += cw
                # pair-0 down-proj for this expert, then pair-1 zero-fill
                bcol(0, j)
                if j < NP:
                    zfill(1, j)

            nc.gpsimd.collective_compute(
                "ReduceScatter", OP.add,
                replica_groups=[list(range(NCORES))],
                ins=[y_p[0][0:T, :]],
                outs=[rs_p[0][:]],
            )

            # ---------------- phase B: pair-1 down-proj + RS ----------------
            for j in range(EPC):
                bcol(1, j)
            nc.gpsimd.collective_compute(
                "ReduceScatter", OP.add,
                replica_groups=[list(range(NCORES))],
                ins=[y_p[1][0:T, :]],
                outs=[rs_p[1][:]],
            )

            # ---------------- phase S: rest of shared intermediate ----------
            for sm in range(S1A, SIT):
                shared_int(sm)

            # ---------------- phase C: shared out + combine with rs ---------
            for n in range(H // NW):
                sh = []
                for hh in range(2):
                    s2 = wstream.tile([128, SIT // 2, NW], FP16, tag="w")
                    nc.scalar.dma_start(
                        s2[:], sw2b[n].rearrange("p (k c) -> p k c", c=NW)
                        [:, hh * (SIT // 2):(hh + 1) * (SIT // 2), :])
                    sh.append(s2)
                for ts in range(TS):
                    po = psum.tile([128, NW], F32, tag="p4a")
                    for k2 in range(SIT):
                        nc.tensor.matmul(po[:], gs[:, k2, ts * 128:(ts + 1) * 128],
                                         sh[k2 // (SIT // 2)][:, k2 % (SIT // 2), :],
                                         start=(k2 == 0), stop=(k2 == SIT - 1))
                    rst = small.tile([128, NW], FP16, tag="rst")
                    nc.sync.dma_start(
                        rst[:],
                        rs_p[n // 2][ts * 128:(ts + 1) * 128,
                                     (n % 2) * NW:(n % 2 + 1) * NW])
                    ott = small.tile([128, NW], FP16, tag="ott")
                    nc.vector.tensor_tensor(ott[:], po[:], rst[:], OP.add)
                    nc.sync.dma_start(
                        out[ts * 128:(ts + 1) * 128, n * NW:(n + 1) * NW], ott[:])

    if compile_:
        nc.compile()
    else:
        nc.insert_library_loads()
    return nc


def host_prep(hidden_states, gate_weight, w1, w2, w3, sw1, sw2, sw3):
    B, S, H = hidden_states.shape
    T = B * S
    E, I = w1.shape[0], w1.shape[1]
    SI = sw1.shape[0]
    EPC = E // NCORES
    KT, MT, SIT = H // 128, I // 128, SI // 128
    N4 = max(H // 512, 1)
    NW = min(H, 512)
    TOUT = T // NCORES

    x = np.ascontiguousarray(hidden_states.reshape(T, H), dtype=np.float32)
    combine = _routing(x, gate_weight.astype(np.float32))
    tok_lists = [np.nonzero(combine[:, e])[0] for e in range(E)]
    counts = np.array([len(t) for t in tok_lists])

    # balance experts over cores: serpentine over count-sorted experts; slot s
    # on every core holds its s-th largest expert, so per-slot capacity is the
    # max over cores of that rank band.
    order = np.argsort(-counts, kind="stable")
    assign = np.zeros((NCORES, EPC), np.int64)
    for s in range(EPC):
        band = order[s * NCORES:(s + 1) * NCORES]
        if s % 2 == 1:
            band = band[::-1]
        for c in range(NCORES):
            assign[c, s] = band[c]
    CAPS = tuple(
        int(max(128, ((counts[assign[:, s]].max() + 127) // 128) * 128))
        for s in range(EPC))
    CAPS16 = tuple(
        int(max(128, ((counts[assign[:, s]].max() + 15) // 16) * 16))
        for s in range(EPC))
    CAP0 = max(CAPS)
    CT0 = CAP0 // 128

    x16 = x.astype(np.float16)
    xT = x.T  # [H, T] view

    s1 = sw1.T.reshape(KT, 128, SIT, 128).transpose(2, 1, 0, 3)
    s3 = sw3.T.reshape(KT, 128, SIT, 128).transpose(2, 1, 0, 3)
    sw13 = np.ascontiguousarray(
        np.concatenate([s1, s3], axis=-1).reshape(SIT, 128, -1), dtype=np.float16)
    sw2b = np.ascontiguousarray(
        sw2.T.reshape(SIT, 128, N4, NW).transpose(2, 1, 0, 3).reshape(N4, 128, -1),
        dtype=np.float16)

    in_maps = []
    for c in range(NCORES):
        els = list(assign[c])
        idx_np = np.zeros((EPC, 128, CAP0 // 16), np.int16)
        idxs_np = np.zeros((EPC, 128, CAP0 // 16), np.int16)
        gat_np = np.zeros((EPC, 128, CT0), np.float32)
        for j, e in enumerate(els):
            cap = CAPS[j]
            toks = tok_lists[e]
            a = np.zeros(cap, np.int16)
            a[:len(toks)] = toks
            idx_np[j, :, :cap // 16] = np.tile(a.reshape(cap // 16, 16).T, (8, 1))
            b2 = np.full(cap, T, np.int16)  # pad rows land on dummy row T
            b2[:len(toks)] = toks
            idxs_np[j, :, :cap // 16] = np.tile(b2.reshape(cap // 16, 16).T, (8, 1))
            gv = np.zeros(cap, np.float32)
            gv[:len(toks)] = combine[toks, e]
            gat_np[j, :, :cap // 128] = gv.reshape(cap // 128, 128).T
        w13c = np.empty((EPC, MT, 128, KT * 256), np.float16)
        w2c = np.empty((EPC, N4, 128, MT * NW), np.float16)
        for j, e in enumerate(els):
            a1 = w1[e].T.reshape(KT, 128, MT, 128).transpose(2, 1, 0, 3)
            a3 = w3[e].T.reshape(KT, 128, MT, 128).transpose(2, 1, 0, 3)
            w13c[j] = np.concatenate([a1, a3], axis=-1).reshape(MT, 128, -1)
            w2c[j] = (w2[e].T.reshape(MT, 128, N4, NW)
                      .transpose(2, 1, 0, 3).reshape(N4, 128, -1))
        xTc = np.ascontiguousarray(
            xT[:, c * TOUT:(c + 1) * TOUT].reshape(KT, 128, TOUT)
            .transpose(1, 0, 2).reshape(128, -1), dtype=np.float16)
        in_maps.append({
            "x16": x16, "xTc": xTc,
            "w13": w13c, "w2b": w2c,
            "sw13": sw13, "sw2b": sw2b,
            "idx": idx_np, "idxs": idxs_np, "gat": gat_np,
        })
    cfg = dict(T=T, H=H, I=I, CAPS=CAPS, CAPS16=CAPS16, SI=SI)
    return in_maps, cfg


def kernel(**inputs):
    inputs = {k: np.asarray(v) for k, v in inputs.items()}
    hs = inputs["hidden_states"]
    B, S, H = hs.shape
    in_maps, cfg = host_prep(
        hs, inputs["gate_weight"], inputs["w1"], inputs["w2"], inputs["w3"],
        inputs["sw1"], inputs["sw2"], inputs["sw3"])
    nc = build_kernel(**cfg)
    res = run_bass_kernel_spmd(nc, in_maps, list(range(NCORES)))
    y = np.concatenate([res.results[c]["out"] for c in range(NCORES)], axis=0)
    return y.reshape(B, S, H).astype(np.float32)


if __name__ == "__main__":
    pass
